# revision 9
# baseline (speedup 1.0000x reference)
"""Trainium2 Bass kernel for nn_LogisticRegression (multi-hot + mean-embedding
logistic regression over a 50k vocab).

Math: for each row i with tokens x[i, 0:200]:
    logit[i] = b + (1/200) * sum_j s[x_ij] + sum_{unique tokens t in row} Wv[t]
    y[i] = sigmoid(logit[i])
where s = E @ w_emb (one scalar per vocab entry), Wv = W[0, 300:].

Device strategy (8 NeuronCores, SPMD):
  - batch-shard rows: 128 rows per core.
  - vocab-shard the s computation: each core streams its 6250-row slice of E
    (host-padded to 6272 rows) and reduces s_shard (DVE mult + ACT accum);
    a 25KB AllGather makes s global.
  - each core builds a value table vt in DRAM: 64B per vocab entry packed as
    [s/200, Wv, 14*pad]; a 256B dma_gather element covers 4 entries.
  - tokens are SORTED per row on DVE via Max8 + MatchReplace (25 rounds);
    sorting preserves multiplicity, so the s-part sums all occurrences while
    the exact unique-token mask for Wv is one adjacent not_equal compare.
  - per-token values come from dma_gather (the MoE gather): idx = x_sorted>>2
    (fits int16), then a 1-of-4 lane select on DVE.
"""
import sys

sys.path.insert(0, "/opt/trn_rl_repo")

import numpy as np

import concourse.bass as bass
import concourse.bacc as bacc
import concourse.mybir as mybir
import concourse.tile as tile

N_CORES = 8
P = 128            # rows per core (batch 1024 / 8)
L = 200            # tokens per row
VOCAB = 50000
EMB = 300
VSH = 6250         # vocab shard per core
VSHP = 6272        # padded E shard rows (49 * 128)
TPP = 49           # E rows per partition (6272 / 128)
NG = 7             # E DMA groups (7 tiles each)
VTOT = 50176       # padded vocab entries in vt (128 * 392)
EPP = VTOT // P    # vt entries per partition (392)
VT_ROW = 16        # f32 per vocab entry in vt (64B)
GB = 4             # vocab entries per 256B gather block
NCHUNK = 4         # gather chunks
JC = L // NCHUNK   # token slots per chunk (50)
F32 = mybir.dt.float32
I32 = mybir.dt.int32
I16 = mybir.dt.int16

_CACHE = {}


def build_nc():
    nc = bacc.Bacc("TRN2", target_bir_lowering=False, debug=True)
    x_in = nc.dram_tensor("x_in", [P, L], I32, kind="ExternalInput")
    e_sh = nc.dram_tensor("e_sh", [VSHP, EMB], F32, kind="ExternalInput")
    wemb = nc.dram_tensor("wemb", [1, EMB], F32, kind="ExternalInput")
    wv_all = nc.dram_tensor("wv_all", [P, EPP], F32, kind="ExternalInput")
    bias_in = nc.dram_tensor("bias_in", [1, 1], F32, kind="ExternalInput")
    y_out = nc.dram_tensor("y_out", [P, 1], F32, kind="ExternalOutput")

    s_shard = nc.dram_tensor("s_shard", [1, VSHP], F32)
    s_full = nc.dram_tensor("s_full", [1, VTOT], F32, addr_space="Shared")
    vt = nc.dram_tensor("vt", [P, EPP * VT_ROW], F32)   # 64B per entry
    blkd = nc.dram_tensor("blkd", [1, P * L], I16)      # wrap bounce

    with tile.TileContext(nc) as tc:
        with (
            tc.tile_pool(name="sb", bufs=1) as pool,
            tc.tile_pool(name="et", bufs=2) as epool,
            tc.tile_pool(name="gat", bufs=2) as gpool,
        ):
            # ---- inputs to SBUF -------------------------------------------
            x_sb = pool.tile([P, L], I32)
            nc.sync.dma_start(out=x_sb[:], in_=x_in[:])
            xf = pool.tile([P, L], F32)
            nc.vector.tensor_copy(out=xf[:], in_=x_sb[:])
            wb = pool.tile([P, EMB], F32)
            nc.sync.dma_start(out=wb[:], in_=wemb[0:1, :].to_broadcast([P, EMB]))
            bb = pool.tile([P, 1], F32)
            nc.sync.dma_start(out=bb[:], in_=bias_in[0:1, :].to_broadcast([P, 1]))

            # ---- vt skeleton: zeros + Wv lanes (independent of s) ---------
            # vt entry v lives at partition v // 392, f32 col 16 * (v % 392).
            vt_sb = pool.tile([P, EPP * VT_ROW], F32)  # 6272 f32/partition
            nc.vector.memset(vt_sb[:], 0.0)
            wv_sb = pool.tile([P, EPP], F32)
            nc.sync.dma_start(out=wv_sb[:], in_=wv_all[:])
            nc.vector.tensor_copy(
                out=vt_sb[:, 1:EPP * VT_ROW:VT_ROW], in_=wv_sb[:]
            )

            # ---- sort each row descending (Max8 + MatchReplace) -----------
            xs = pool.tile([P, L], F32)       # sorted tokens
            work = pool.tile([P, L], F32)
            nc.vector.tensor_copy(out=work[:], in_=xf[:])
            for k in range(L // 8):
                nc.vector.max(out=xs[:, 8 * k:8 * k + 8], in_=work[:])
                nc.vector.match_replace(
                    out=work[:],
                    in_to_replace=xs[:, 8 * k:8 * k + 8],
                    in_values=work[:],
                    imm_value=-1.0,
                )
            # unique-token weight: 1 at the first slot of each equal-run
            wdup = pool.tile([P, L], F32)
            nc.vector.memset(wdup[:, 0:1], 1.0)
            nc.vector.tensor_tensor(
                out=wdup[:, 1:L], in0=xs[:, 1:L], in1=xs[:, 0:L - 1],
                op=mybir.AluOpType.not_equal,
            )

            # ---- device index math on sorted tokens -----------------------
            xi = pool.tile([P, L], I32)
            nc.vector.tensor_copy(out=xi[:], in_=xs[:])
            blk32 = pool.tile([P, L], I32)
            nc.vector.tensor_scalar(
                out=blk32[:], in0=xi[:], scalar1=2, scalar2=None,
                op0=mybir.AluOpType.arith_shift_right,
            )
            blk16 = pool.tile([P, L], I16)
            nc.vector.tensor_copy(out=blk16[:], in_=blk32[:])
            mm32 = pool.tile([P, L], I32)
            nc.vector.tensor_scalar(
                out=mm32[:], in0=xi[:], scalar1=3, scalar2=None,
                op0=mybir.AluOpType.bitwise_and,
            )
            mmf = pool.tile([P, L], F32)
            nc.vector.tensor_copy(out=mmf[:], in_=mm32[:])
            # lane masks m in 0..3, plain (for s) and dedup-weighted (for Wv)
            m4 = pool.tile([P, L * GB], F32)
            m4w = pool.tile([P, L * GB], F32)
            for m in range(GB):
                nc.vector.tensor_scalar(
                    out=m4[:, m * L:(m + 1) * L], in0=mmf[:],
                    scalar1=float(m), scalar2=None,
                    op0=mybir.AluOpType.is_equal,
                )
                nc.vector.tensor_tensor(
                    out=m4w[:, m * L:(m + 1) * L],
                    in0=m4[:, m * L:(m + 1) * L], in1=wdup[:],
                    op=mybir.AluOpType.mult,
                )

            # ---- wrap blk16 into dma_gather idx layout via DRAM bounce ----
            # idx k = j*128 + row  ->  wrapped slot (k%16, k//16), replicated
            # to all 8 partition groups.
            nc.sync.dma_start(
                out=blkd[0].rearrange("(j r) -> r j", r=P), in_=blk16[:]
            )
            idx_sb = pool.tile([P, L * 8], I16)
            for g in range(8):
                nc.sync.dma_start(
                    out=idx_sb[16 * g:16 * (g + 1), :],
                    in_=blkd[0].rearrange("(s q) -> q s", q=16),
                )

            # ---- stream E shard; s = (E @ wemb) / L -----------------------
            s_sb = pool.tile([P, TPP], F32)
            prod = pool.tile([P, EMB], F32)
            for g in range(NG):
                et = epool.tile([P, NG * EMB], F32, tag="et")
                nc.sync.dma_start(
                    out=et[:],
                    in_=e_sh[:].rearrange("(a b) e -> a b e", a=P)[
                        :, g * NG:(g + 1) * NG, :
                    ],
                )
                for u in range(NG):
                    t = g * NG + u
                    nc.vector.tensor_scalar(
                        out=prod[:], in0=et[:, u * EMB:(u + 1) * EMB],
                        scalar1=1.0 / L, scalar2=None,
                        op0=mybir.AluOpType.mult,
                    )
                    nc.vector.tensor_tensor(
                        out=prod[:], in0=prod[:], in1=wb[:],
                        op=mybir.AluOpType.mult,
                    )
                    nc.scalar.activation(
                        out=prod[:], in_=prod[:],
                        func=mybir.ActivationFunctionType.Copy,
                        scale=1.0, accum_out=s_sb[:, t:t + 1],
                    )
            # s_shard flat: value (p, t) -> local entry 49p + t
            nc.sync.dma_start(
                out=s_shard[0].rearrange("(a b) -> a b", a=P), in_=s_sb[:]
            )

            # ---- zero vt pad zone of s_full, then AllGather ----------------
            zpad = pool.tile([1, VTOT - VOCAB], F32)
            nc.vector.memset(zpad[:], 0.0)
            nc.sync.dma_start(out=s_full[0:1, VOCAB:VTOT], in_=zpad[:])
            nc.gpsimd.collective_compute(
                "AllGather", mybir.AluOpType.bypass,
                replica_groups=[list(range(N_CORES))],
                ins=[s_shard[0:1, 0:VSH]],
                outs=[s_full[0:1, 0:VOCAB].rearrange("o (a b) -> (o a) b", a=N_CORES)],
            )

            # ---- finish vt: interleave s lanes, write to DRAM -------------
            sf_sb = pool.tile([P, EPP], F32)
            nc.sync.dma_start(
                out=sf_sb[:], in_=s_full[0].rearrange("(p f) -> p f", p=P)
            )
            nc.vector.tensor_copy(
                out=vt_sb[:, 0:EPP * VT_ROW:VT_ROW], in_=sf_sb[:]
            )
            nc.sync.dma_start(out=vt[:], in_=vt_sb[:])

            # ---- gather values: 4 chunks of 6400 idx ----------------------
            accs = pool.tile([P, 2 * GB * NCHUNK], F32)  # 32 partial columns
            scrg = pool.tile([P, JC], F32)
            vt_rows = vt[:].rearrange("a b -> (a b)").rearrange(
                "(r e) -> r e", e=64
            )
            for t in range(NCHUNK):
                gout = gpool.tile([P, JC * 64], F32, tag="g")
                nc.gpsimd.dma_gather(
                    out_ap=gout[:].rearrange("p (j e) -> p j e", e=64),
                    in_ap=vt_rows,
                    idxs_ap=idx_sb[:, t * 8 * JC:(t + 1) * 8 * JC],
                    num_idxs=P * JC,
                    num_idxs_reg=P * JC,
                    elem_size=64,
                    elem_step=64,
                    single_packet=False,
                )
                g3 = gout[:].rearrange("p (j e) -> p j e", e=64)
                for m in range(GB):
                    col = t * 8 + m
                    nc.vector.tensor_tensor(
                        out=scrg[:], in0=g3[:, :, m * VT_ROW],
                        in1=m4[:, m * L + t * JC: m * L + (t + 1) * JC],
                        op=mybir.AluOpType.mult,
                    )
                    nc.vector.tensor_reduce(
                        out=accs[:, col:col + 1], in_=scrg[:],
                        axis=mybir.AxisListType.X, op=mybir.AluOpType.add,
                    )
                    nc.vector.tensor_tensor(
                        out=scrg[:], in0=g3[:, :, m * VT_ROW + 1],
                        in1=m4w[:, m * L + t * JC: m * L + (t + 1) * JC],
                        op=mybir.AluOpType.mult,
                    )
                    nc.vector.tensor_reduce(
                        out=accs[:, col + 4:col + 5], in_=scrg[:],
                        axis=mybir.AxisListType.X, op=mybir.AluOpType.add,
                    )

            # ---- logit + sigmoid ------------------------------------------
            pre = pool.tile([P, 1], F32)
            nc.vector.tensor_reduce(
                out=pre[:], in_=accs[:],
                axis=mybir.AxisListType.X, op=mybir.AluOpType.add,
            )
            y_sb = pool.tile([P, 1], F32)
            nc.scalar.activation(
                out=y_sb[:], in_=pre[:],
                func=mybir.ActivationFunctionType.Sigmoid,
                bias=bb[:, 0:1], scale=1.0,
            )
            nc.sync.dma_start(out=y_out[:], in_=y_sb[:])
    nc.compile()
    return nc


def prep_inputs(x, embedding_weight, W, b):
    """Host-side sharding/layout prep. Returns per-core input maps."""
    x = np.asarray(x)
    E = np.asarray(embedding_weight, dtype=np.float32)
    W = np.asarray(W, dtype=np.float32)
    b = np.asarray(b, dtype=np.float32)
    wemb = W[0, :EMB].reshape(1, EMB).copy()
    wv_pad = np.zeros(VTOT, dtype=np.float32)
    wv_pad[:VOCAB] = W[0, EMB:]
    wv_all = wv_pad.reshape(P, EPP)
    xi = x.astype(np.int32)

    in_maps = []
    for c in range(N_CORES):
        e_c = np.zeros((VSHP, EMB), dtype=np.float32)
        e_c[:VSH] = E[c * VSH:(c + 1) * VSH]
        in_maps.append({
            "x_in": xi[c * P:(c + 1) * P],
            "e_sh": e_c,
            "wemb": wemb,
            "wv_all": wv_all,
            "bias_in": b.reshape(1, 1),
        })
    return in_maps


def kernel(**inputs):
    if "nc" not in _CACHE:
        _CACHE["nc"] = build_nc()
    nc = _CACHE["nc"]
    in_maps = prep_inputs(**inputs)
    from concourse.bass_utils import run_bass_kernel_spmd
    r = run_bass_kernel_spmd(nc, in_maps, list(range(N_CORES)))
    y = np.concatenate([r.results[c]["y_out"] for c in range(N_CORES)], axis=0)
    return y.astype(np.float32)


# revision 11
# speedup vs baseline: 2.5180x; 2.5180x over previous
"""Trainium2 Bass kernel for nn_LogisticRegression (multi-hot + mean-embedding
logistic regression over a 50k vocab).

Math: for each row i with tokens x[i, 0:200]:
    logit[i] = b + (1/200) * sum_j s[x_ij] + sum_{unique tokens t in row} Wv[t]
    y[i] = sigmoid(logit[i])
where s = E @ w_emb (one scalar per vocab entry), Wv = W[0, 300:].

Device strategy (8 NeuronCores, SPMD):
  - batch-shard rows: 128 rows per core.
  - vocab-shard the s computation: each core streams its 6250-row slice of E
    (host-padded to 6272 rows) and reduces s_shard (DVE mult + ACT accum);
    a 25KB AllGather makes s global.
  - each core builds a value table vt in DRAM: 64B per vocab entry packed as
    [s/200, Wv, 14*pad]; a 256B dma_gather element covers 4 entries.
  - tokens are SORTED per row on DVE via Max8 + MatchReplace (25 rounds);
    sorting preserves multiplicity, so the s-part sums all occurrences while
    the exact unique-token mask for Wv is one adjacent not_equal compare.
  - per-token values come from dma_gather (the MoE gather): idx = x_sorted>>2
    (fits int16), then a 1-of-4 lane select on DVE.
"""
import sys

sys.path.insert(0, "/opt/trn_rl_repo")

import numpy as np

import concourse.bass as bass
import concourse.bacc as bacc
import concourse.mybir as mybir
import concourse.tile as tile

N_CORES = 8
P = 128            # rows per core (batch 1024 / 8)
L = 200            # tokens per row
VOCAB = 50000
EMB = 300
VSH = 6250         # vocab shard per core
VSHP = 6272        # padded E shard rows (49 * 128)
TPP = 49           # E rows per partition (6272 / 128)
NG = 7             # E DMA groups (7 tiles each)
VTOT = 50176       # padded vocab entries in vt (128 * 392)
EPP = VTOT // P    # vt entries per partition (392)
VT_ROW = 16        # f32 per vocab entry in vt (64B)
GB = 4             # vocab entries per 256B gather block
NCHUNK = 4         # gather chunks
JC = L // NCHUNK   # token slots per chunk (50)
F32 = mybir.dt.float32
I32 = mybir.dt.int32
I16 = mybir.dt.int16

_CACHE = {}


def build_nc():
    nc = bacc.Bacc("TRN2", target_bir_lowering=False, debug=True)
    x_in = nc.dram_tensor("x_in", [P, L], I32, kind="ExternalInput")
    e_sh = nc.dram_tensor("e_sh", [VSHP, EMB], F32, kind="ExternalInput")
    wemb = nc.dram_tensor("wemb", [1, EMB], F32, kind="ExternalInput")
    wv_all = nc.dram_tensor("wv_all", [P, EPP], F32, kind="ExternalInput")
    bias_in = nc.dram_tensor("bias_in", [1, 1], F32, kind="ExternalInput")
    y_out = nc.dram_tensor("y_out", [P, 1], F32, kind="ExternalOutput")

    s_shard = nc.dram_tensor("s_shard", [1, VSHP], F32)
    s_full = nc.dram_tensor("s_full", [1, VTOT], F32, addr_space="Shared")
    vt = nc.dram_tensor("vt", [P, EPP * VT_ROW], F32)   # 64B per entry

    with tile.TileContext(nc) as tc:
        with (
            tc.tile_pool(name="sb", bufs=1) as pool,
            tc.tile_pool(name="et", bufs=2) as epool,
            tc.tile_pool(name="gat", bufs=2) as gpool,
        ):
            # ---- inputs to SBUF -------------------------------------------
            x_sb = pool.tile([P, L], I32)
            nc.sync.dma_start(out=x_sb[:], in_=x_in[:])
            xf = pool.tile([P, L], F32)
            nc.vector.tensor_copy(out=xf[:], in_=x_sb[:])
            wb = pool.tile([P, EMB], F32)
            nc.sync.dma_start(out=wb[:], in_=wemb[0:1, :].to_broadcast([P, EMB]))
            bb = pool.tile([P, 1], F32)
            nc.sync.dma_start(out=bb[:], in_=bias_in[0:1, :].to_broadcast([P, 1]))

            # ---- vt skeleton: zeros + Wv lanes (independent of s) ---------
            # vt entry v lives at partition v // 392, f32 col 16 * (v % 392).
            vt_sb = pool.tile([P, EPP * VT_ROW], F32)  # 6272 f32/partition
            nc.vector.memset(vt_sb[:], 0.0)
            wv_sb = pool.tile([P, EPP], F32)
            nc.sync.dma_start(out=wv_sb[:], in_=wv_all[:])
            nc.vector.tensor_copy(
                out=vt_sb[:, 1:EPP * VT_ROW:VT_ROW], in_=wv_sb[:]
            )

            # ---- sort each row descending (Max8 + MatchReplace) -----------
            xs = pool.tile([P, L], F32)       # sorted tokens
            work = pool.tile([P, L], F32)
            nc.vector.tensor_copy(out=work[:], in_=xf[:])
            for k in range(L // 8):
                nc.vector.max(out=xs[:, 8 * k:8 * k + 8], in_=work[:])
                nc.vector.match_replace(
                    out=work[:],
                    in_to_replace=xs[:, 8 * k:8 * k + 8],
                    in_values=work[:],
                    imm_value=-1.0,
                )
            # unique-token weight: 1 at the first slot of each equal-run
            wdup = pool.tile([P, L], F32)
            nc.vector.memset(wdup[:, 0:1], 1.0)
            nc.vector.tensor_tensor(
                out=wdup[:, 1:L], in0=xs[:, 1:L], in1=xs[:, 0:L - 1],
                op=mybir.AluOpType.not_equal,
            )

            # ---- device index math on sorted tokens -----------------------
            xi = pool.tile([P, L], I32)
            nc.vector.tensor_copy(out=xi[:], in_=xs[:])
            blk32 = pool.tile([P, L], I32)
            nc.vector.tensor_scalar(
                out=blk32[:], in0=xi[:], scalar1=2, scalar2=None,
                op0=mybir.AluOpType.arith_shift_right,
            )
            blk16 = pool.tile([P, L], I16)
            nc.vector.tensor_copy(out=blk16[:], in_=blk32[:])
            mm32 = pool.tile([P, L], I32)
            nc.vector.tensor_scalar(
                out=mm32[:], in0=xi[:], scalar1=3, scalar2=None,
                op0=mybir.AluOpType.bitwise_and,
            )
            mmf = pool.tile([P, L], F32)
            nc.vector.tensor_copy(out=mmf[:], in_=mm32[:])
            # lane masks m in 0..3, plain (for s) and dedup-weighted (for Wv)
            m4 = pool.tile([P, L * GB], F32)
            m4w = pool.tile([P, L * GB], F32)
            for m in range(GB):
                nc.vector.tensor_scalar(
                    out=m4[:, m * L:(m + 1) * L], in0=mmf[:],
                    scalar1=float(m), scalar2=None,
                    op0=mybir.AluOpType.is_equal,
                )
                nc.vector.tensor_tensor(
                    out=m4w[:, m * L:(m + 1) * L],
                    in0=m4[:, m * L:(m + 1) * L], in1=wdup[:],
                    op=mybir.AluOpType.mult,
                )

            # ---- wrap blk16 into dma_gather idx layout -------------------
            # idx k = j*128 + row -> wrapped slot (row%16, 8j + row//16),
            # replicated to all 8 partition groups. Partition fold via 8
            # partition-shift SBUF DMAs, then DVE stride-8 interleave, then
            # group replication (all contiguous descriptors).
            tmpw = pool.tile([16, L * 8], I16)   # [pp, 200q + j]
            for q in range(8):
                nc.sync.dma_start(
                    out=tmpw[:, L * q:L * (q + 1)],
                    in_=blk16[16 * q:16 * (q + 1), :],
                )
            idx0 = pool.tile([16, L * 8], I16)   # [pp, 8j + q]
            for q in range(8):
                nc.vector.tensor_copy(
                    out=idx0[:, q:L * 8:8], in_=tmpw[:, L * q:L * (q + 1)]
                )
            idx_sb = pool.tile([P, L * 8], I16)
            for g in range(8):
                nc.sync.dma_start(
                    out=idx_sb[16 * g:16 * (g + 1), :], in_=idx0[:]
                )

            # ---- stream E shard; s = (E @ wemb) / L -----------------------
            s_sb = pool.tile([P, TPP], F32)
            prod = pool.tile([P, EMB], F32)
            for g in range(NG):
                et = epool.tile([P, NG * EMB], F32, tag="et")
                nc.sync.dma_start(
                    out=et[:],
                    in_=e_sh[:].rearrange("(a b) e -> a b e", a=P)[
                        :, g * NG:(g + 1) * NG, :
                    ],
                )
                for u in range(NG):
                    t = g * NG + u
                    nc.vector.tensor_scalar(
                        out=prod[:], in0=et[:, u * EMB:(u + 1) * EMB],
                        scalar1=1.0 / L, scalar2=None,
                        op0=mybir.AluOpType.mult,
                    )
                    nc.vector.tensor_tensor(
                        out=prod[:], in0=prod[:], in1=wb[:],
                        op=mybir.AluOpType.mult,
                    )
                    nc.scalar.activation(
                        out=prod[:], in_=prod[:],
                        func=mybir.ActivationFunctionType.Copy,
                        scale=1.0, accum_out=s_sb[:, t:t + 1],
                    )
            # s_shard flat: value (p, t) -> local entry 49p + t
            nc.sync.dma_start(
                out=s_shard[0].rearrange("(a b) -> a b", a=P), in_=s_sb[:]
            )

            # ---- zero vt pad zone of s_full, then AllGather ----------------
            zpad = pool.tile([1, VTOT - VOCAB], F32)
            nc.vector.memset(zpad[:], 0.0)
            nc.sync.dma_start(out=s_full[0:1, VOCAB:VTOT], in_=zpad[:])
            nc.gpsimd.collective_compute(
                "AllGather", mybir.AluOpType.bypass,
                replica_groups=[list(range(N_CORES))],
                ins=[s_shard[0:1, 0:VSH]],
                outs=[s_full[0:1, 0:VOCAB].rearrange("o (a b) -> (o a) b", a=N_CORES)],
            )

            # ---- finish vt: interleave s lanes, write to DRAM -------------
            sf_sb = pool.tile([P, EPP], F32)
            nc.sync.dma_start(
                out=sf_sb[:], in_=s_full[0].rearrange("(p f) -> p f", p=P)
            )
            nc.vector.tensor_copy(
                out=vt_sb[:, 0:EPP * VT_ROW:VT_ROW], in_=sf_sb[:]
            )
            nc.sync.dma_start(out=vt[:], in_=vt_sb[:])

            # ---- gather values: 4 chunks of 6400 idx ----------------------
            accs = pool.tile([P, 2 * GB * NCHUNK], F32)  # 32 partial columns
            scrg = pool.tile([P, JC], F32)
            vt_rows = vt[:].rearrange("a b -> (a b)").rearrange(
                "(r e) -> r e", e=64
            )
            for t in range(NCHUNK):
                gout = gpool.tile([P, JC * 64], F32, tag="g")
                nc.gpsimd.dma_gather(
                    out_ap=gout[:].rearrange("p (j e) -> p j e", e=64),
                    in_ap=vt_rows,
                    idxs_ap=idx_sb[:, t * 8 * JC:(t + 1) * 8 * JC],
                    num_idxs=P * JC,
                    num_idxs_reg=P * JC,
                    elem_size=64,
                    elem_step=64,
                    single_packet=False,
                )
                g3 = gout[:].rearrange("p (j e) -> p j e", e=64)
                for m in range(GB):
                    col = t * 8 + m
                    nc.vector.tensor_tensor(
                        out=scrg[:], in0=g3[:, :, m * VT_ROW],
                        in1=m4[:, m * L + t * JC: m * L + (t + 1) * JC],
                        op=mybir.AluOpType.mult,
                    )
                    nc.vector.tensor_reduce(
                        out=accs[:, col:col + 1], in_=scrg[:],
                        axis=mybir.AxisListType.X, op=mybir.AluOpType.add,
                    )
                    nc.vector.tensor_tensor(
                        out=scrg[:], in0=g3[:, :, m * VT_ROW + 1],
                        in1=m4w[:, m * L + t * JC: m * L + (t + 1) * JC],
                        op=mybir.AluOpType.mult,
                    )
                    nc.vector.tensor_reduce(
                        out=accs[:, col + 4:col + 5], in_=scrg[:],
                        axis=mybir.AxisListType.X, op=mybir.AluOpType.add,
                    )

            # ---- logit + sigmoid ------------------------------------------
            pre = pool.tile([P, 1], F32)
            nc.vector.tensor_reduce(
                out=pre[:], in_=accs[:],
                axis=mybir.AxisListType.X, op=mybir.AluOpType.add,
            )
            y_sb = pool.tile([P, 1], F32)
            nc.scalar.activation(
                out=y_sb[:], in_=pre[:],
                func=mybir.ActivationFunctionType.Sigmoid,
                bias=bb[:, 0:1], scale=1.0,
            )
            nc.sync.dma_start(out=y_out[:], in_=y_sb[:])
    nc.compile()
    return nc


def prep_inputs(x, embedding_weight, W, b):
    """Host-side sharding/layout prep. Returns per-core input maps."""
    x = np.asarray(x)
    E = np.asarray(embedding_weight, dtype=np.float32)
    W = np.asarray(W, dtype=np.float32)
    b = np.asarray(b, dtype=np.float32)
    wemb = W[0, :EMB].reshape(1, EMB).copy()
    wv_pad = np.zeros(VTOT, dtype=np.float32)
    wv_pad[:VOCAB] = W[0, EMB:]
    wv_all = wv_pad.reshape(P, EPP)
    xi = x.astype(np.int32)

    in_maps = []
    for c in range(N_CORES):
        e_c = np.zeros((VSHP, EMB), dtype=np.float32)
        e_c[:VSH] = E[c * VSH:(c + 1) * VSH]
        in_maps.append({
            "x_in": xi[c * P:(c + 1) * P],
            "e_sh": e_c,
            "wemb": wemb,
            "wv_all": wv_all,
            "bias_in": b.reshape(1, 1),
        })
    return in_maps


def kernel(**inputs):
    if "nc" not in _CACHE:
        _CACHE["nc"] = build_nc()
    nc = _CACHE["nc"]
    in_maps = prep_inputs(**inputs)
    from concourse.bass_utils import run_bass_kernel_spmd
    r = run_bass_kernel_spmd(nc, in_maps, list(range(N_CORES)))
    y = np.concatenate([r.results[c]["y_out"] for c in range(N_CORES)], axis=0)
    return y.astype(np.float32)
